# revision 15
# baseline (speedup 1.0000x reference)
"""GCN (3x GCNConv + global_mean_pool + linear) on 8 Trainium2 NeuronCores.

Self-contained: hardcoded problem shapes (N=50000, E=800000, H=128, F_IN=11,
G=2048).

Math (per conv layer, PyG GCNConv):
    z[d] = dinv[d] * ( sum_{e:dst=d} dinv[src_e]*x[src_e]  +  dinv[d]*x[d] )
    x' = relu(z @ W + b)          (no relu on layer 3)
with dinv = 1/sqrt(1+indeg). The feature table is pre-scaled by dinv (x~ =
dinv*x) so edge contributions need only the dst-side dinv, applied per
128-node block after PSUM accumulation.

Distribution: nodes (padded to 50176 = 8*49*128) sharded contiguously across
8 cores; each core aggregates its own dst blocks, gathering source rows from
a replicated feature table (AllGather per layer). Pooling partials are
scatter-written to graph rows and AllReduced.

Device pipeline per layer/core (per dst block b, tails pipelined one block
behind the aggregation matmuls):
  dma_gather src rows (512-desc chunks, f32-typed 256B rows carrying bf16
  payload - "V16") -> PE: pzN[dst,feat] += M_tile^T @ G_tile (lhsT = host-
  precomputed one-hot M loaded from DRAM via HWDGE, rhs = gathered tile
  bitcast bf16) + an identity matmul adds the node-major self-loop tile ->
  ACT copy pzN*dinv -> PE transpose -> ACT copy -> PE @W -> ACT relu+bias ->
  PE transpose -> ACT copy*dinv into the next layer's self tiles -> DMA to
  shard -> AllGather.

Critical performance facts (HW-measured):
- SWDGE descriptor generation bounds the kernel. 4 SWDGE queues
  (num_swdge_queues=4, queue_num round-robin) run it at ~1.8 ns/row vs ~8-10
  on one queue. <=512 descriptors per dma_gather, ~2 in flight per queue.
- bf16-typed gathers are SLOWER per descriptor than f32, hence the "bf16
  payload viewed as f32 half-width" trick with bitcast at the PE.
- ANY 2-input DVE op locks the SBUF port pair GpSimd needs to write SWDGE
  descriptors (exclusive lock, full block) - overlapped DVE work stretches
  the gathers ~3x. The hot path must use only PE/ACT/DMA: M matrices are
  host-precomputed and DMA-loaded; all per-block scaling uses ACT
  `activation(scale=per-partition dinv)` in node-major layout; pooling
  accumulates in PSUM. DVE appears only in startup consts and the tiny
  per-inference epilogue.
- The 8-core AllGather costs only tens of us; don't bother splitting it.
"""
import sys

sys.path.insert(0, "/opt/trn_rl_repo")

import numpy as np

N_NODES = 50000
N_EDGES = 800000
HIDDEN = 128
F_IN = 11
F1 = 128                   # layer-1 feature padding (bf16 -> 256B rows)
NUM_CLASSES = 19
NUM_GRAPHS = 2048
NCORES = 8
BLK = 128
NBLK = 49                  # blocks per core
SHARD = NBLK * BLK         # 6272 nodes per core
NPAD = NCORES * SHARD      # 50176
TW = F1 // 2               # table row width in f32 elements (bf16 payload)
LO_END = 17408             # A-window: table[0:32768), idx=src
HI_BASE = NPAD - 32768     # 17408; B-window: table[17408:50176), idx=src-HI_BASE
ACC_S = 512                # core-relative pooling slots (4 tiles of 128)
PD_ROWS = 2176             # padded graph rows for scatter (>=2048, *19 %128==0)

_cache = {}


# --------------------------------------------------------------------------
# host preprocessing
# --------------------------------------------------------------------------
def _preprocess(x, edge_index, batch, W1, b1, W2, b2, W3, b3, Wl, bl):
    import ml_dtypes
    bfnp = ml_dtypes.bfloat16

    src = np.asarray(edge_index[0], dtype=np.int64)
    dst = np.asarray(edge_index[1], dtype=np.int64)
    batch = np.asarray(batch, dtype=np.int64)
    x = np.asarray(x, np.float32)

    x_pad = np.zeros((NPAD, F_IN), np.float32)
    x_pad[:N_NODES] = x
    batch_pad = np.full(NPAD, -1, np.int64)
    batch_pad[:N_NODES] = batch

    # --- in-degree-balanced node permutation within 12-block windows -------
    # (keeps pooling graph-windows narrow while equalizing per-block edge
    #  counts so the uniform SPMD tile budgets waste fewer gather slots)
    indeg = np.bincount(dst, minlength=NPAD).astype(np.int64)
    indeg_lo = np.bincount(dst[src < LO_END], minlength=NPAD).astype(np.int64)
    perm = np.arange(NPAD)
    import os
    W = 12
    for c in range(NCORES if os.environ.get("GCN_BAL", "1") == "1" else 0):
        for w0 in range(0, NBLK, W):
            nb = min(W, NBLK - w0)
            p0 = c * SHARD + w0 * BLK
            ids = perm[p0:p0 + nb * BLK].copy()
            tot, lo = indeg[ids], indeg_lo[ids]
            at = max(tot.sum() / nb, 1.0)
            al = max(lo.sum() / nb, 1.0)
            order = np.argsort(-tot, kind="stable")
            bt = np.zeros(nb)
            blo = np.zeros(nb)
            bcnt = np.zeros(nb, np.int64)
            assign = np.empty(nb * BLK, np.int64)
            for i in order:
                scr = np.maximum((bt + tot[i]) / at, (blo + lo[i]) / al)
                scr[bcnt >= BLK] = np.inf
                b = int(np.argmin(scr))
                assign[i] = b
                bt[b] += tot[i]
                blo[b] += lo[i]
                bcnt[b] += 1
            perm[p0:p0 + nb * BLK] = np.concatenate(
                [ids[assign == b] for b in range(nb)])
    inv = np.empty(NPAD, np.int64)
    inv[perm] = np.arange(NPAD)
    src = inv[src]
    dst = inv[dst]
    x_pad = x_pad[perm]
    batch_pad = batch_pad[perm]

    deg = 1.0 + np.bincount(dst, minlength=NPAD).astype(np.float32)
    dinv_pad = (1.0 / np.sqrt(deg)).astype(np.float32)
    # per-node dinv laid out [node-in-block, NBLK] per core
    dinv_nb = np.ascontiguousarray(
        dinv_pad.reshape(NCORES, NBLK, BLK).transpose(0, 2, 1))

    xs = (x_pad * dinv_pad[:, None]).astype(np.float32)   # x~ = dinv * x
    # bf16 payload disguised as f32 rows of width F1//2: gather moves 256B
    # rows at the (faster) f32 descriptor rate; PE reads via bitcast(bf16)
    t1b = np.zeros((NPAD, F1), bfnp)
    t1b[:, :F_IN] = xs.astype(bfnp)
    table1 = t1b.view(np.float32)                         # [NPAD, TW]
    # layer-1 self-loop tiles, node-major per shard (same bf16-as-f32 view)
    self1 = np.ascontiguousarray(
        t1b.view(np.float32).reshape(NCORES, SHARD, TW))  # [C, SHARD, TW]

    # --- edge grouping -----------------------------------------------------
    core_of = dst // SHARD
    blk_of = (dst % SHARD) // BLK
    rel_of = (dst % BLK).astype(np.int64)
    gblk = core_of * NBLK + blk_of
    cls = np.where(src < LO_END, 0, np.where(src >= 32768, 2, 1)).astype(np.int8)

    nblk_g = NCORES * NBLK
    n_lo = np.bincount(gblk[cls == 0], minlength=nblk_g)
    n_mid = np.bincount(gblk[cls == 1], minlength=nblk_g)
    n_hi = np.bincount(gblk[cls == 2], minlength=nblk_g)

    T_A = max(1, int(np.max(-(-n_lo // BLK))))
    a_fill = np.minimum(n_mid, T_A * BLK - n_lo)
    T_B = max(1, int(np.max(-(-(n_hi + n_mid - a_fill) // BLK))))
    ntile = T_A + T_B
    slots_core = NBLK * ntile * BLK

    order = np.lexsort((cls, gblk))
    src_o, rel_o, cls_o = src[order], rel_of[order], cls[order]
    blk_starts = np.searchsorted(gblk[order], np.arange(nblk_g + 1))

    idx_all = np.zeros((NCORES, slots_core), np.int16)
    dstrel_all = np.full((NCORES, slots_core), 255, np.int64)
    for c in range(NCORES):
        for run in (0, 1):
            T_r = T_A if run == 0 else T_B
            base0 = 0 if run == 0 else NBLK * T_A * BLK
            for b in range(NBLK):
                g = c * NBLK + b
                s, e = blk_starts[g], blk_starts[g + 1]
                bsrc, brel, bcls = src_o[s:e], rel_o[s:e], cls_o[s:e]
                a = int(a_fill[g])
                mid_idx = np.nonzero(bcls == 1)[0]
                if run == 0:
                    sel = np.concatenate([np.nonzero(bcls == 0)[0], mid_idx[:a]])
                    iv = bsrc[sel]
                else:
                    sel = np.concatenate([mid_idx[a:], np.nonzero(bcls == 2)[0]])
                    iv = bsrc[sel] - HI_BASE
                k = len(sel)
                assert k <= T_r * BLK
                pos = base0 + b * T_r * BLK
                idx_all[c, pos:pos + k] = iv.astype(np.int16)
                dstrel_all[c, pos:pos + k] = brel[sel]

    idx16 = np.zeros((NCORES, 128, slots_core // 16), np.int16)
    for c in range(NCORES):
        idx16[c] = np.tile(idx_all[c].reshape(-1, 16).T, (8, 1))

    # host-precomputed one-hot M, per block: [NBLK, 128 slot, ntile, 128 dst]
    # (fp8 e4m3 raw bytes, 0x38 == 1.0; loaded per block as the aggregation
    # matmuls' lhsT — mixed fp8 lhsT x bf16 rhs is a supported PE mode and
    # one-hot values are exact in fp8, at half the HBM traffic of bf16)
    mall = np.zeros((NCORES, NBLK, 128, ntile, 128), np.uint8)
    for c in range(NCORES):
        arrA = dstrel_all[c][:NBLK * T_A * BLK].reshape(NBLK, T_A, BLK)
        arrB = dstrel_all[c][NBLK * T_A * BLK:].reshape(NBLK, T_B, BLK)
        for arr, t0 in ((arrA, 0), (arrB, T_A)):
            bi, ti, pi = np.nonzero(arr <= 127)
            mall[c, bi, pi, t0 + ti, arr[bi, ti, pi]] = 0x38

    # --- pooling -----------------------------------------------------------
    cnt = np.bincount(batch, minlength=NUM_GRAPHS).astype(np.float32)
    inv_cnt = (1.0 / np.maximum(cnt, 1.0)).astype(np.float32)
    bp = batch_pad.reshape(NCORES, SHARD)
    gc_lo = np.array([int(bp[c][bp[c] >= 0].min()) for c in range(NCORES)])

    # uniform (SPMD) core-relative window base per block: cover all cores
    lo_need = np.full(NBLK, 10 ** 9, np.int64)
    hi_need = np.full(NBLK, 0, np.int64)
    for c in range(NCORES):
        for b in range(NBLK):
            nodes = bp[c, b * BLK:(b + 1) * BLK]
            real = nodes[nodes >= 0]
            if len(real):
                lo_need[b] = min(lo_need[b], real.min() - gc_lo[c])
                hi_need[b] = max(hi_need[b], real.max() - gc_lo[c])
    u_of = np.clip(lo_need, 0, ACC_S - BLK)
    assert (hi_need - u_of).max() < BLK and hi_need.max() < ACC_S

    Bmat = np.zeros((NCORES, 128, NBLK * BLK), np.float32)
    for c in range(NCORES):
        for b in range(NBLK):
            nodes = bp[c, b * BLK:(b + 1) * BLK]
            p = np.nonzero(nodes >= 0)[0]
            if len(p) == 0:
                continue
            s = nodes[p] - gc_lo[c] - u_of[b]
            assert (s >= 0).all() and (s < BLK).all(), (c, b, s.min(), s.max())
            Bmat[c, p, b * BLK + s] = inv_cnt[nodes[p]]

    # absolute graph row per core-relative slot; dummies -> pad rows
    gidx = np.zeros((NCORES, 128, 4), np.int32)
    covered = np.zeros((NCORES, ACC_S), bool)
    for c in range(NCORES):
        for k in range(4):
            g_abs = gc_lo[c] + k * 128 + np.arange(128)
            ok = g_abs < NUM_GRAPHS
            gidx[c, :, k] = np.where(ok, g_abs, 2100)
            covered[c, k * 128:(k + 1) * 128] = ok

    # bias: designate exactly one (core, slot) per graph
    biasmat = np.zeros((NCORES, NUM_CLASSES, ACC_S), np.float32)
    bl32 = np.asarray(bl, np.float32)
    done = np.zeros(NUM_GRAPHS, bool)
    for c in range(NCORES):
        for sl in range(ACC_S):
            if covered[c, sl]:
                g = gc_lo[c] + sl
                if not done[g]:
                    done[g] = True
                    biasmat[c, :, sl] = bl32
    assert done.all()

    W1pf = np.zeros((F1, HIDDEN), np.float32)  # rows >= F_IN stay zero
    W1pf[:F_IN] = np.asarray(W1, np.float32)
    wts = dict(
        W1p=W1pf.astype(bfnp).view(np.float32),
        W2=np.asarray(W2, np.float32).astype(bfnp).view(np.float32),
        W3=np.asarray(W3, np.float32).astype(bfnp).view(np.float32),
        Wl=np.asarray(Wl, np.float32),
        identm=np.eye(128, dtype=bfnp).view(np.float32),
        b1=np.asarray(b1, np.float32).reshape(HIDDEN, 1),
        b2=np.asarray(b2, np.float32).reshape(HIDDEN, 1),
        b3=np.asarray(b3, np.float32).reshape(HIDDEN, 1),
    )

    meta = dict(T_A=T_A, T_B=T_B, slots_core=slots_core, u_of=u_of)
    per_core = [dict(idx16=idx16[c], mall=mall[c].reshape(NBLK, -1),
                     self1=self1[c], dinv_nb=dinv_nb[c], Bmat=Bmat[c],
                     gidx=gidx[c], biasmat=biasmat[c], table1=table1, **wts)
                for c in range(NCORES)]
    return meta, per_core


# --------------------------------------------------------------------------
# device program
# --------------------------------------------------------------------------
def _build(meta, repeat=1):
    import concourse.bacc as bacc
    import concourse.bass as bass
    import concourse.tile as tile
    from concourse import mybir
    from concourse.masks import make_identity

    T_A, T_B = meta["T_A"], meta["T_B"]
    slots = meta["slots_core"]
    ntile = T_A + T_B
    f32 = mybir.dt.float32
    bf16 = mybir.dt.bfloat16
    f8 = mybir.dt.float8e4
    u8 = mybir.dt.uint8

    import os
    scr = int(os.environ.get("GCN_SCR", "32768"))
    nswq = int(os.environ.get("GCN_NSWQ", "4"))
    nc = bacc.Bacc("TRN2", target_bir_lowering=False, debug=False,
                   num_devices=NCORES, dynamic_dma_scratch_size=scr,
                   num_swdge_queues=nswq)
    ti = lambda n, s, d=f32: nc.dram_tensor(n, s, d, kind="ExternalInput")
    table1 = ti("table1", [NPAD, TW])
    idx16 = ti("idx16", [128, slots // 16], mybir.dt.int16)
    mall_d = ti("mall", [NBLK, 128 * ntile * 128], u8)
    self1_d = ti("self1", [SHARD, TW])
    dinv_d = ti("dinv_nb", [128, NBLK])
    Bmat_d = ti("Bmat", [128, NBLK * BLK])
    gidx_d = ti("gidx", [128, 4], mybir.dt.int32)
    biasmat_d = ti("biasmat", [NUM_CLASSES, ACC_S])
    W1p_d = ti("W1p", [F1, HIDDEN // 2])
    W2_d = ti("W2", [HIDDEN, HIDDEN // 2])
    W3_d = ti("W3", [HIDDEN, HIDDEN // 2])
    identm_d = ti("identm", [128, 64])
    Wl_d = ti("Wl", [HIDDEN, NUM_CLASSES])
    b1_d, b2_d, b3_d = ti("b1", [HIDDEN, 1]), ti("b2", [HIDDEN, 1]), ti("b3", [HIDDEN, 1])
    out_d = nc.dram_tensor("out", [NUM_GRAPHS, NUM_CLASSES], f32,
                           kind="ExternalOutput")

    with tile.TileContext(nc) as tc:
        with (
            tc.tile_pool(name="const", bufs=1) as cp,
            tc.tile_pool(name="work", bufs=1) as wp,
            tc.tile_pool(name="ps", bufs=2, space="PSUM") as ps,
            tc.tile_pool(name="dram", bufs=1, space="DRAM") as dp,
        ):
            # ---- constants / persistent state ----
            idx_sb = cp.tile([128, slots // 16], mybir.dt.int16)
            nc.sync.dma_start(idx_sb[:], idx16[:])
            dinv_nb = cp.tile([128, NBLK], f32)
            nc.sync.dma_start(dinv_nb[:], dinv_d[:])
            # self-loop tiles, node-major bf16: ping-pong across layers
            selfA = cp.tile([128, NBLK, 2 * TW], bf16)
            nc.sync.dma_start(
                selfA[:].bitcast(f32),
                self1_d[:].rearrange("(b p) f -> p b f", p=128))
            selfB = cp.tile([128, NBLK, 2 * TW], bf16)
            ident = cp.tile([128, 128], f32)
            make_identity(nc, ident[:])
            identm = cp.tile([128, 128], bf16)
            nc.sync.dma_start(identm[:].bitcast(f32), identm_d[:])

            def load_w16(dram, shape, tg):
                # distinct tag per weight: same-tag cp tiles share one slot
                # ring (bufs=1), which deadlocks the scheduler when reps>1
                # re-read an early weight after its slot was recycled
                wb = cp.tile(shape, bf16, tag=tg, name=tg)
                nc.sync.dma_start(wb[:].bitcast(f32), dram[:])
                return wb

            W1p = load_w16(W1p_d, [F1, HIDDEN], "w1p")
            W2 = load_w16(W2_d, [HIDDEN, HIDDEN], "w2")
            W3 = load_w16(W3_d, [HIDDEN, HIDDEN], "w3")
            Wl = cp.tile([HIDDEN, NUM_CLASSES], f32)
            nc.sync.dma_start(Wl[:], Wl_d[:])
            b1 = cp.tile([HIDDEN, 1], f32)
            nc.sync.dma_start(b1[:], b1_d[:])
            b2 = cp.tile([HIDDEN, 1], f32)
            nc.sync.dma_start(b2[:], b2_d[:])
            b3 = cp.tile([HIDDEN, 1], f32)
            nc.sync.dma_start(b3[:], b3_d[:])

            u_of = meta["u_of"]

            # gather chunk in tiles of 128 descriptors; 4 SWDGE queues
            # round-robin, ~2 in flight per queue
            CH = int(os.environ.get("GCN_CH", "8"))
            NSWQ = nswq
            GBUFS = int(os.environ.get("GCN_GBUFS", "8"))
            MLOOK = int(os.environ.get("GCN_MLOOK", "2"))
            MRES = int(os.environ.get("GCN_MRES", "20"))   # M blocks resident
            MBUFS = int(os.environ.get("GCN_MBUFS", "6"))  # streamed-M ring
            NOAG = os.environ.get("GCN_NOAG", "0") == "1"    # ablation
            NOGAT = os.environ.get("GCN_NOGAT", "0") == "1"  # ablation
            qctr = [0]

            # M blocks [0, MRES) stay SBUF-resident for the whole inference
            # (M is layer-invariant): loaded once, reused 3x per rep, and
            # layer starts after an AllGather don't wait on M DMA.
            if MRES > 0:
                mres = cp.tile([128, MRES, ntile, 128], f8)
                nc.sync.dma_start(
                    mres[:].bitcast(u8),
                    mall_d[0:MRES].rearrange("b (p t d) -> p b t d", p=128,
                                             t=ntile))

            # streamed M ring, shared across layers so the next layer's
            # tiles can prefetch during the previous layer's AllGather
            mstream = {}

            def load_mb(lnum, b):
                if b >= NBLK or b < MRES or (lnum, b) in mstream:
                    return
                t = wp.tile([128, ntile, 128], f8, tag="mb",
                            bufs=MBUFS, name=f"mb_{lnum}_{b}")
                nc.sync.dma_start(
                    t[:].bitcast(u8),
                    mall_d[b].rearrange("(p t d) -> p t d", p=128,
                                        t=ntile))
                mstream[(lnum, b)] = t

            def mb_ap(lnum, b):
                if b < MRES:
                    return mres[:, b]
                return mstream[(lnum, b)][:]

            def layer(lnum, tbl, W_sb, b_sb, s_cur, s_nxt, ag_in, ag_out,
                      acc_ps, nxt_lnum=None):
                role = (lnum - 1) % 3 + 1
                nA, nB = NBLK * T_A, NBLK * T_B
                aCH = [(s, min(s + CH, nA)) for s in range(0, nA, CH)]
                bCH = [(s, min(s + CH, nB)) for s in range(0, nB, CH)]
                ga, gb = {}, {}
                ai = bi = 0

                def tail(b, pz):
                    # node-major tail: ACT + PE only (no DVE - it would lock
                    # GpSimd out of the SBUF ports SWDGE needs)
                    dv = dinv_nb[:, b:b + 1]
                    ztN = wp.tile([128, HIDDEN], bf16, tag="ztN", bufs=2,
                                  name=f"ztN_{lnum}_{b}")
                    nc.scalar.activation(ztN[:], pz[:],
                                         mybir.ActivationFunctionType.Copy,
                                         scale=dv)
                    ztT = ps.tile([HIDDEN, 128], bf16, tag="pt16", bufs=2,
                                  name=f"ztT_{lnum}_{b}")
                    nc.tensor.transpose(ztT[:], ztN[:], identm[:])
                    ztF = wp.tile([HIDDEN, 128], bf16, tag="ztF", bufs=2,
                                  name=f"ztF_{lnum}_{b}")
                    nc.scalar.copy(ztF[:], ztT[:])
                    pxn = ps.tile([HIDDEN, 128], f32, tag="pz", bufs=3,
                                  name=f"pxn_{lnum}_{b}")
                    nc.tensor.matmul(pxn[:], lhsT=W_sb[:], rhs=ztF[:],
                                     start=True, stop=True)
                    if role < 3:
                        xh = wp.tile([HIDDEN, 128], bf16, tag="xh", bufs=2,
                                     name=f"xh_{lnum}_{b}")
                        nc.scalar.activation(xh[:], pxn[:],
                                             mybir.ActivationFunctionType.Relu,
                                             bias=b_sb[:])
                        ptr = ps.tile([128, HIDDEN], bf16, tag="pt16", bufs=2,
                                      name=f"ptr_{lnum}_{b}")
                        nc.tensor.transpose(ptr[:], xh[:], identm[:])
                        nc.scalar.activation(s_nxt[:, b, :], ptr[:],
                                             mybir.ActivationFunctionType.Copy,
                                             scale=dv)
                        nc.sync.dma_start(
                            ag_in[b * BLK:(b + 1) * BLK, :],
                            s_nxt[:, b, :].bitcast(f32))
                    else:
                        h3 = wp.tile([HIDDEN, 128], f32, tag="xh", bufs=2,
                                     name=f"h3_{b}")
                        nc.scalar.activation(h3[:], pxn[:],
                                             mybir.ActivationFunctionType.Identity,
                                             bias=b_sb[:])
                        ptr = ps.tile([128, HIDDEN], f32, tag="ptr", bufs=2,
                                      name=f"ptr3_{b}")
                        nc.tensor.transpose(ptr[:], h3[:], ident[:])
                        tr = wp.tile([128, HIDDEN], f32, tag="tr", bufs=2,
                                     name=f"tr3_{b}")
                        nc.scalar.copy(tr[:], ptr[:])
                        bt = wp.tile([128, BLK], f32, tag="bt", bufs=4,
                                     name=f"bt_{b}")
                        nc.sync.dma_start(bt[:],
                                          Bmat_d[:, b * BLK:(b + 1) * BLK])
                        u = int(u_of[b])
                        nc.tensor.matmul(acc_ps[:, u:u + BLK], lhsT=tr[:],
                                         rhs=bt[:], start=False, stop=True)

                for b0 in range(MLOOK):
                    load_mb(lnum, MRES + b0)
                pending = None
                for b in range(NBLK):
                    load_mb(lnum, b + MLOOK if b + MLOOK >= MRES else MRES)
                    while ai < len(aCH) and aCH[ai][0] < (b + 1) * T_A:
                        s, e = aCH[ai]
                        gt = wp.tile([128, e - s, TW], f32, tag="gA",
                                     bufs=GBUFS, name=f"gA_{lnum}_{ai}")
                        if not NOGAT:
                            nc.gpsimd.dma_gather(
                                gt[:], tbl[0:32768, :], idx_sb[:, s * 8:e * 8],
                                (e - s) * BLK, (e - s) * BLK, TW,
                                queue_num=qctr[0] % NSWQ)
                        qctr[0] += 1
                        ga[ai] = gt
                        ai += 1
                    while bi < len(bCH) and bCH[bi][0] < (b + 1) * T_B:
                        s, e = bCH[bi]
                        gt = wp.tile([128, e - s, TW], f32, tag="gB",
                                     bufs=GBUFS, name=f"gB_{lnum}_{bi}")
                        if not NOGAT:
                            nc.gpsimd.dma_gather(
                                gt[:], tbl[HI_BASE:NPAD, :],
                                idx_sb[:, nA * 8 + s * 8:nA * 8 + e * 8],
                                (e - s) * BLK, (e - s) * BLK, TW,
                                queue_num=qctr[0] % NSWQ)
                        qctr[0] += 1
                        gb[bi] = gt
                        bi += 1
                    pz = ps.tile([128, HIDDEN], f32, tag="pz", bufs=3,
                                 name=f"pz_{lnum}_{b}")
                    nt = 0
                    for run, gmap, T_r, col0 in (
                        (0, ga, T_A, b * T_A),
                        (1, gb, T_B, b * T_B),
                    ):
                        for t in range(T_r):
                            j = col0 + t                  # stream tile index
                            chunk, sl = j // CH, j % CH
                            nc.tensor.matmul(
                                pz[:], lhsT=mb_ap(lnum, b)[:, nt, :],
                                rhs=gmap[chunk][:].bitcast(bf16)[:, sl, :],
                                start=(nt == 0), stop=False)
                            nt += 1
                    # self-loop: pz[d, f] += self[d, f] via identity lhsT
                    nc.tensor.matmul(pz[:], lhsT=identm[:],
                                     rhs=s_cur[:, b, :],
                                     start=False, stop=True)
                    if pending is not None:
                        tail(*pending)
                    pending = (b, pz)
                if pending is not None:
                    tail(*pending)

                if role < 3:
                    # prefetch the next layer's streamed M into the ring so
                    # those DMAs overlap the AllGather instead of serializing
                    # after it
                    if nxt_lnum is not None:
                        for pb in range(MRES, min(MRES + MBUFS - 1, NBLK)):
                            load_mb(nxt_lnum, pb)
                    if NOAG:
                        nc.sync.dma_start(ag_out[0:SHARD, :], ag_in[:])
                    else:
                        nc.gpsimd.collective_compute(
                            "AllGather", mybir.AluOpType.bypass,
                            replica_groups=[list(range(NCORES))],
                            ins=[ag_in[:]], outs=[ag_out[:]])

            ag_in1 = dp.tile([SHARD, TW], f32)
            ag_in2 = dp.tile([SHARD, TW], f32)
            gidx_sb = cp.tile([128, 4], mybir.dt.int32)
            nc.sync.dma_start(gidx_sb[:], gidx_d[:])
            biasm_sb = cp.tile([NUM_CLASSES, ACC_S], f32)
            nc.sync.dma_start(biasm_sb[:], biasmat_d[:])
            pd_ab = [dp.tile([PD_ROWS, NUM_CLASSES], f32, tag=f"pd{i}",
                             name=f"pd{i}") for i in range(2)]
            zt19 = wp.tile([128, PD_ROWS * NUM_CLASSES // 128], f32)
            nc.vector.memset(zt19[:], 0.0)

            for rep in range(repeat):
              ln1, ln2, ln3 = 3 * rep + 1, 3 * rep + 2, 3 * rep + 3
              ag1_out = dp.tile([NPAD, TW], f32, addr_space="Shared",
                                name=f"ag1_out_{rep}", tag=f"ag1_{rep}")
              ag2_out = dp.tile([NPAD, TW], f32, addr_space="Shared",
                                name=f"ag2_out_{rep}", tag=f"ag2_{rep}")
              # pooling accumulator lives in PSUM; the layer-3 tail matmuls
              # accumulate into it (start=False), so zero it first (the
              # early-epilogue accT copy frees it well before the next rep)
              acc_ps = ps.tile([128, ACC_S], f32, tag="acc", bufs=1,
                               name=f"accps_{rep}")
              nc.vector.memset(acc_ps[:], 0.0)
              # alternate pooling scatter buffers so rep r+1's zeroing does
              # not wait on rep r's AllReduce read (WAR)
              pd = pd_ab[rep % 2]
              layer(ln1, table1, W1p, b1, selfA, selfB, ag_in1, ag1_out, None,
                    nxt_lnum=ln2)
              layer(ln2, ag1_out, W2, b2, selfB, selfA, ag_in2, ag2_out, None,
                    nxt_lnum=ln3)
              layer(ln3, ag2_out, W3, b3, selfA, selfB, None, None, acc_ps)

              # ---- pooling epilogue (gathers are done; DVE is safe) ----
              nc.sync.dma_start(
                pd[:].rearrange("a b -> (a b)").rearrange("(p f) -> p f", p=128),
                zt19[:])
              accT = wp.tile([128, ACC_S], f32, tag="accsb", bufs=2,
                             name=f"accsb_{rep}")
              nc.scalar.copy(accT[:], acc_ps[:])

              for k in range(4):
                py = ps.tile([NUM_CLASSES, 128], f32, tag="ptr", bufs=2,
                             name=f"py_{rep}_{k}")
                nc.tensor.matmul(py[:], lhsT=Wl[:],
                                 rhs=accT[:, k * 128:(k + 1) * 128],
                                 start=True, stop=True)
                y = wp.tile([NUM_CLASSES, 128], f32, tag="ye", bufs=2,
                            name=f"y_{rep}_{k}")
                nc.vector.tensor_tensor(
                    out=y[:], in0=py[:],
                    in1=biasm_sb[:, k * 128:(k + 1) * 128],
                    op=mybir.AluOpType.add)
                pyt = ps.tile([128, NUM_CLASSES], f32, tag="ptr", bufs=2,
                              name=f"pyt_{rep}_{k}")
                nc.tensor.transpose(pyt[:], y[:],
                                    ident[:NUM_CLASSES, :NUM_CLASSES])
                yT = wp.tile([128, NUM_CLASSES], f32, tag="yt2", bufs=2,
                             name=f"yT_{rep}_{k}")
                nc.scalar.copy(yT[:], pyt[:])
                nc.gpsimd.indirect_dma_start(
                    out=pd[:],
                    out_offset=bass.IndirectOffsetOnAxis(ap=gidx_sb[:, k:k + 1],
                                                         axis=0),
                    in_=yT[:], in_offset=None)

              pd_red = dp.tile([PD_ROWS, NUM_CLASSES], f32,
                               addr_space="Shared", name=f"pd_red_{rep}",
                               tag=f"pdr_{rep}")
              nc.gpsimd.collective_compute(
                "AllReduce", mybir.AluOpType.add,
                replica_groups=[list(range(NCORES))],
                ins=[pd[:]], outs=[pd_red[:]])
              nc.sync.dma_start(out_d[:], pd_red[0:NUM_GRAPHS, :])

    nc.compile()
    return nc


# --------------------------------------------------------------------------
def kernel(**inputs):
    from concourse import bass_utils

    meta, per_core = _preprocess(**inputs)
    key = (meta["T_A"], meta["T_B"])
    if key not in _cache:
        _cache[key] = _build(meta)
    nc = _cache[key]
    res = bass_utils.run_bass_kernel_spmd(nc, per_core,
                                          core_ids=list(range(NCORES)))
    return np.asarray(res.results[0]["out"], np.float32)



# revision 22
# speedup vs baseline: 1.0965x; 1.0965x over previous
"""GCN (3x GCNConv + global_mean_pool + linear) on 8 Trainium2 NeuronCores.

Self-contained: hardcoded problem shapes (N=50000, E=800000, H=128, F_IN=11,
G=2048).

Math (per conv layer, PyG GCNConv):
    z[d] = dinv[d] * ( sum_{e:dst=d} dinv[src_e]*x[src_e]  +  dinv[d]*x[d] )
    x' = relu(z @ W + b)          (no relu on layer 3)
with dinv = 1/sqrt(1+indeg). The feature table is pre-scaled by dinv (x~ =
dinv*x) so edge contributions need only the dst-side dinv, applied per
128-node block after PSUM accumulation.

Distribution: nodes (padded to 50176 = 8*49*128) sharded contiguously across
8 cores; each core aggregates its own dst blocks, gathering source rows from
a replicated feature table (AllGather per layer). Pooling partials are
scatter-written to graph rows and AllReduced.

Device pipeline per layer/core (per dst block b, tails pipelined one block
behind the aggregation matmuls):
  dma_gather src rows (512-desc chunks, f32-typed 256B rows carrying bf16
  payload - "V16") -> PE: pzN[dst,feat] += M_tile^T @ G_tile (lhsT = host-
  precomputed one-hot M loaded from DRAM via HWDGE, rhs = gathered tile
  bitcast bf16) + an identity matmul adds the node-major self-loop tile ->
  ACT copy pzN*dinv -> PE transpose -> ACT copy -> PE @W -> ACT relu+bias ->
  PE transpose -> ACT copy*dinv into the next layer's self tiles -> DMA to
  shard -> AllGather.

Critical performance facts (HW-measured):
- SWDGE descriptor generation bounds the kernel. 4 SWDGE queues
  (num_swdge_queues=4, queue_num round-robin) run it at ~1.8 ns/row vs ~8-10
  on one queue. <=512 descriptors per dma_gather, ~2 in flight per queue.
- bf16-typed gathers are SLOWER per descriptor than f32, hence the "bf16
  payload viewed as f32 half-width" trick with bitcast at the PE.
- ANY 2-input DVE op locks the SBUF port pair GpSimd needs to write SWDGE
  descriptors (exclusive lock, full block) - overlapped DVE work stretches
  the gathers ~3x. The hot path must use only PE/ACT/DMA: M matrices are
  host-precomputed and DMA-loaded; all per-block scaling uses ACT
  `activation(scale=per-partition dinv)` in node-major layout; pooling
  accumulates in PSUM. DVE appears only in startup consts and the tiny
  per-inference epilogue.
- The 8-core AllGather costs only tens of us; don't bother splitting it.
"""
import sys

sys.path.insert(0, "/opt/trn_rl_repo")

import numpy as np

N_NODES = 50000
N_EDGES = 800000
HIDDEN = 128
F_IN = 11
F1 = 128                   # layer-1 feature padding (bf16 -> 256B rows)
NUM_CLASSES = 19
NUM_GRAPHS = 2048
NCORES = 8
BLK = 128
NBLK = 49                  # blocks per core
SHARD = NBLK * BLK         # 6272 nodes per core
NPAD = NCORES * SHARD      # 50176
TW = F1 // 2               # table row width in f32 elements (bf16 payload)
LO_END = 17408             # A-window: table[0:32768), idx=src
HI_BASE = NPAD - 32768     # 17408; B-window: table[17408:50176), idx=src-HI_BASE
ACC_S = 512                # core-relative pooling slots (4 tiles of 128)
PD_ROWS = 2176             # padded graph rows for scatter (>=2048, *19 %128==0)

_cache = {}


# --------------------------------------------------------------------------
# host preprocessing
# --------------------------------------------------------------------------
def _preprocess(x, edge_index, batch, W1, b1, W2, b2, W3, b3, Wl, bl):
    import ml_dtypes
    bfnp = ml_dtypes.bfloat16

    src = np.asarray(edge_index[0], dtype=np.int64)
    dst = np.asarray(edge_index[1], dtype=np.int64)
    batch = np.asarray(batch, dtype=np.int64)
    x = np.asarray(x, np.float32)

    x_pad = np.zeros((NPAD, F_IN), np.float32)
    x_pad[:N_NODES] = x
    batch_pad = np.full(NPAD, -1, np.int64)
    batch_pad[:N_NODES] = batch

    # --- in-degree-balanced node permutation within 12-block windows -------
    # (keeps pooling graph-windows narrow while equalizing per-block edge
    #  counts so the uniform SPMD tile budgets waste fewer gather slots)
    indeg = np.bincount(dst, minlength=NPAD).astype(np.int64)
    indeg_lo = np.bincount(dst[src < LO_END], minlength=NPAD).astype(np.int64)
    indeg_hi = np.bincount(dst[src >= 32768], minlength=NPAD).astype(np.int64)
    perm = np.arange(NPAD)
    import os
    W = int(os.environ.get("GCN_BALW", "12"))
    for c in range(NCORES if os.environ.get("GCN_BAL", "1") == "1" else 0):
        for w0 in range(0, NBLK, W):
            nb = min(W, NBLK - w0)
            p0 = c * SHARD + w0 * BLK
            ids = perm[p0:p0 + nb * BLK].copy()
            tot, lo, hi = indeg[ids], indeg_lo[ids], indeg_hi[ids]
            at = max(tot.sum() / nb, 1.0)
            al = max(lo.sum() / nb, 1.0)
            ah = max(hi.sum() / nb, 1.0)
            order = np.argsort(-tot, kind="stable")
            bt = np.zeros(nb)
            blo = np.zeros(nb)
            bhi = np.zeros(nb)
            bcnt = np.zeros(nb, np.int64)
            assign = np.empty(nb * BLK, np.int64)
            for i in order:
                scr = np.maximum(
                    np.maximum((bt + tot[i]) / at, (blo + lo[i]) / al),
                    (bhi + hi[i]) / ah)
                scr[bcnt >= BLK] = np.inf
                b = int(np.argmin(scr))
                assign[i] = b
                bt[b] += tot[i]
                blo[b] += lo[i]
                bhi[b] += hi[i]
                bcnt[b] += 1
            perm[p0:p0 + nb * BLK] = np.concatenate(
                [ids[assign == b] for b in range(nb)])
    inv = np.empty(NPAD, np.int64)
    inv[perm] = np.arange(NPAD)
    src = inv[src]
    dst = inv[dst]
    x_pad = x_pad[perm]
    batch_pad = batch_pad[perm]

    deg = 1.0 + np.bincount(dst, minlength=NPAD).astype(np.float32)
    dinv_pad = (1.0 / np.sqrt(deg)).astype(np.float32)
    # per-node dinv laid out [node-in-block, NBLK] per core
    dinv_nb = np.ascontiguousarray(
        dinv_pad.reshape(NCORES, NBLK, BLK).transpose(0, 2, 1))

    xs = (x_pad * dinv_pad[:, None]).astype(np.float32)   # x~ = dinv * x
    # bf16 payload disguised as f32 rows of width F1//2: gather moves 256B
    # rows at the (faster) f32 descriptor rate; PE reads via bitcast(bf16)
    t1b = np.zeros((NPAD, F1), bfnp)
    t1b[:, :F_IN] = xs.astype(bfnp)
    table1 = t1b.view(np.float32)                         # [NPAD, TW]
    # layer-1 self-loop tiles, node-major per shard (same bf16-as-f32 view)
    self1 = np.ascontiguousarray(
        t1b.view(np.float32).reshape(NCORES, SHARD, TW))  # [C, SHARD, TW]

    # --- edge grouping -----------------------------------------------------
    core_of = dst // SHARD
    blk_of = (dst % SHARD) // BLK
    rel_of = (dst % BLK).astype(np.int64)
    gblk = core_of * NBLK + blk_of
    cls = np.where(src < LO_END, 0, np.where(src >= 32768, 2, 1)).astype(np.int8)

    nblk_g = NCORES * NBLK
    n_lo = np.bincount(gblk[cls == 0], minlength=nblk_g)
    n_mid = np.bincount(gblk[cls == 1], minlength=nblk_g)
    n_hi = np.bincount(gblk[cls == 2], minlength=nblk_g)

    T_A = max(1, int(np.max(-(-n_lo // BLK))))
    a_fill = np.minimum(n_mid, T_A * BLK - n_lo)
    T_B = max(1, int(np.max(-(-(n_hi + n_mid - a_fill) // BLK))))
    ntile = T_A + T_B
    slots_core = NBLK * ntile * BLK

    order = np.lexsort((cls, gblk))
    src_o, rel_o, cls_o = src[order], rel_of[order], cls[order]
    blk_starts = np.searchsorted(gblk[order], np.arange(nblk_g + 1))

    idx_all = np.zeros((NCORES, slots_core), np.int16)
    dstrel_all = np.full((NCORES, slots_core), 255, np.int64)
    for c in range(NCORES):
        for run in (0, 1):
            T_r = T_A if run == 0 else T_B
            base0 = 0 if run == 0 else NBLK * T_A * BLK
            for b in range(NBLK):
                g = c * NBLK + b
                s, e = blk_starts[g], blk_starts[g + 1]
                bsrc, brel, bcls = src_o[s:e], rel_o[s:e], cls_o[s:e]
                a = int(a_fill[g])
                mid_idx = np.nonzero(bcls == 1)[0]
                if run == 0:
                    sel = np.concatenate([np.nonzero(bcls == 0)[0], mid_idx[:a]])
                    iv = bsrc[sel]
                else:
                    sel = np.concatenate([mid_idx[a:], np.nonzero(bcls == 2)[0]])
                    iv = bsrc[sel] - HI_BASE
                k = len(sel)
                assert k <= T_r * BLK
                pos = base0 + b * T_r * BLK
                idx_all[c, pos:pos + k] = iv.astype(np.int16)
                dstrel_all[c, pos:pos + k] = brel[sel]

    idx16 = np.zeros((NCORES, 128, slots_core // 16), np.int16)
    for c in range(NCORES):
        idx16[c] = np.tile(idx_all[c].reshape(-1, 16).T, (8, 1))

    # host-precomputed one-hot M, per block: [NBLK, 128 slot, ntile, 128 dst]
    # (fp8 e4m3 raw bytes, 0x38 == 1.0; loaded per block as the aggregation
    # matmuls' lhsT — mixed fp8 lhsT x bf16 rhs is a supported PE mode and
    # one-hot values are exact in fp8, at half the HBM traffic of bf16)
    mall = np.zeros((NCORES, NBLK, 128, ntile, 128), np.uint8)
    for c in range(NCORES):
        arrA = dstrel_all[c][:NBLK * T_A * BLK].reshape(NBLK, T_A, BLK)
        arrB = dstrel_all[c][NBLK * T_A * BLK:].reshape(NBLK, T_B, BLK)
        for arr, t0 in ((arrA, 0), (arrB, T_A)):
            bi, ti, pi = np.nonzero(arr <= 127)
            mall[c, bi, pi, t0 + ti, arr[bi, ti, pi]] = 0x38

    # --- pooling -----------------------------------------------------------
    cnt = np.bincount(batch, minlength=NUM_GRAPHS).astype(np.float32)
    inv_cnt = (1.0 / np.maximum(cnt, 1.0)).astype(np.float32)
    bp = batch_pad.reshape(NCORES, SHARD)
    gc_lo = np.array([int(bp[c][bp[c] >= 0].min()) for c in range(NCORES)])

    # uniform (SPMD) core-relative window base per block: cover all cores
    lo_need = np.full(NBLK, 10 ** 9, np.int64)
    hi_need = np.full(NBLK, 0, np.int64)
    for c in range(NCORES):
        for b in range(NBLK):
            nodes = bp[c, b * BLK:(b + 1) * BLK]
            real = nodes[nodes >= 0]
            if len(real):
                lo_need[b] = min(lo_need[b], real.min() - gc_lo[c])
                hi_need[b] = max(hi_need[b], real.max() - gc_lo[c])
    u_of = np.clip(lo_need, 0, ACC_S - BLK)
    assert (hi_need - u_of).max() < BLK and hi_need.max() < ACC_S

    Bmat = np.zeros((NCORES, 128, NBLK * BLK), np.float32)
    for c in range(NCORES):
        for b in range(NBLK):
            nodes = bp[c, b * BLK:(b + 1) * BLK]
            p = np.nonzero(nodes >= 0)[0]
            if len(p) == 0:
                continue
            s = nodes[p] - gc_lo[c] - u_of[b]
            assert (s >= 0).all() and (s < BLK).all(), (c, b, s.min(), s.max())
            Bmat[c, p, b * BLK + s] = inv_cnt[nodes[p]]

    # absolute graph row per core-relative slot; dummies -> pad rows
    gidx = np.zeros((NCORES, 128, 4), np.int32)
    covered = np.zeros((NCORES, ACC_S), bool)
    for c in range(NCORES):
        for k in range(4):
            g_abs = gc_lo[c] + k * 128 + np.arange(128)
            ok = g_abs < NUM_GRAPHS
            gidx[c, :, k] = np.where(ok, g_abs, 2100)
            covered[c, k * 128:(k + 1) * 128] = ok

    # bias: designate exactly one (core, slot) per graph
    biasmat = np.zeros((NCORES, NUM_CLASSES, ACC_S), np.float32)
    bl32 = np.asarray(bl, np.float32)
    done = np.zeros(NUM_GRAPHS, bool)
    for c in range(NCORES):
        for sl in range(ACC_S):
            if covered[c, sl]:
                g = gc_lo[c] + sl
                if not done[g]:
                    done[g] = True
                    biasmat[c, :, sl] = bl32
    assert done.all()

    W1pf = np.zeros((F1, HIDDEN), np.float32)  # rows >= F_IN stay zero
    W1pf[:F_IN] = np.asarray(W1, np.float32)
    wts = dict(
        W1p=W1pf.astype(bfnp).view(np.float32),
        W2=np.asarray(W2, np.float32).astype(bfnp).view(np.float32),
        W3=np.asarray(W3, np.float32).astype(bfnp).view(np.float32),
        Wl=np.asarray(Wl, np.float32),
        identm=np.eye(128, dtype=bfnp).view(np.float32),
        b1=np.asarray(b1, np.float32).reshape(HIDDEN, 1),
        b2=np.asarray(b2, np.float32).reshape(HIDDEN, 1),
        b3=np.asarray(b3, np.float32).reshape(HIDDEN, 1),
    )

    meta = dict(T_A=T_A, T_B=T_B, slots_core=slots_core, u_of=u_of)
    per_core = [dict(idx16=idx16[c], mall=mall[c].reshape(NBLK, -1),
                     self1=self1[c], dinv_nb=dinv_nb[c], Bmat=Bmat[c],
                     gidx=gidx[c], biasmat=biasmat[c], table1=table1, **wts)
                for c in range(NCORES)]
    return meta, per_core


# --------------------------------------------------------------------------
# device program
# --------------------------------------------------------------------------
def _build(meta, repeat=1):
    import concourse.bacc as bacc
    import concourse.bass as bass
    import concourse.tile as tile
    from concourse import mybir
    from concourse.masks import make_identity

    T_A, T_B = meta["T_A"], meta["T_B"]
    slots = meta["slots_core"]
    ntile = T_A + T_B
    f32 = mybir.dt.float32
    bf16 = mybir.dt.bfloat16
    f8 = mybir.dt.float8e4
    u8 = mybir.dt.uint8

    import os
    scr = int(os.environ.get("GCN_SCR", "32768"))
    nswq = int(os.environ.get("GCN_NSWQ", "4"))
    nc = bacc.Bacc("TRN2", target_bir_lowering=False, debug=False,
                   num_devices=NCORES, dynamic_dma_scratch_size=scr,
                   num_swdge_queues=nswq)
    ti = lambda n, s, d=f32: nc.dram_tensor(n, s, d, kind="ExternalInput")
    table1 = ti("table1", [NPAD, TW])
    idx16 = ti("idx16", [128, slots // 16], mybir.dt.int16)
    mall_d = ti("mall", [NBLK, 128 * ntile * 128], u8)
    self1_d = ti("self1", [SHARD, TW])
    dinv_d = ti("dinv_nb", [128, NBLK])
    Bmat_d = ti("Bmat", [128, NBLK * BLK])
    gidx_d = ti("gidx", [128, 4], mybir.dt.int32)
    biasmat_d = ti("biasmat", [NUM_CLASSES, ACC_S])
    W1p_d = ti("W1p", [F1, HIDDEN // 2])
    W2_d = ti("W2", [HIDDEN, HIDDEN // 2])
    W3_d = ti("W3", [HIDDEN, HIDDEN // 2])
    identm_d = ti("identm", [128, 64])
    Wl_d = ti("Wl", [HIDDEN, NUM_CLASSES])
    b1_d, b2_d, b3_d = ti("b1", [HIDDEN, 1]), ti("b2", [HIDDEN, 1]), ti("b3", [HIDDEN, 1])
    out_d = nc.dram_tensor("out", [NUM_GRAPHS, NUM_CLASSES], f32,
                           kind="ExternalOutput")

    with tile.TileContext(nc) as tc:
        with (
            tc.tile_pool(name="const", bufs=1) as cp,
            tc.tile_pool(name="work", bufs=1) as wp,
            tc.tile_pool(name="ps", bufs=2, space="PSUM") as ps,
            tc.tile_pool(name="dram", bufs=1, space="DRAM") as dp,
        ):
            # ---- constants / persistent state ----
            idx_sb = cp.tile([128, slots // 16], mybir.dt.int16)
            nc.sync.dma_start(idx_sb[:], idx16[:])
            dinv_nb = cp.tile([128, NBLK], f32)
            nc.sync.dma_start(dinv_nb[:], dinv_d[:])
            # self-loop tiles, node-major bf16: ping-pong across layers
            selfA = cp.tile([128, NBLK, 2 * TW], bf16)
            nc.sync.dma_start(
                selfA[:].bitcast(f32),
                self1_d[:].rearrange("(b p) f -> p b f", p=128))
            selfB = cp.tile([128, NBLK, 2 * TW], bf16)
            ident = cp.tile([128, 128], f32)
            make_identity(nc, ident[:])
            identm = cp.tile([128, 128], bf16)
            nc.sync.dma_start(identm[:].bitcast(f32), identm_d[:])

            def load_w16(dram, shape, tg):
                # distinct tag per weight: same-tag cp tiles share one slot
                # ring (bufs=1), which deadlocks the scheduler when reps>1
                # re-read an early weight after its slot was recycled
                wb = cp.tile(shape, bf16, tag=tg, name=tg)
                nc.sync.dma_start(wb[:].bitcast(f32), dram[:])
                return wb

            W1p = load_w16(W1p_d, [F1, HIDDEN], "w1p")
            W2 = load_w16(W2_d, [HIDDEN, HIDDEN], "w2")
            W3 = load_w16(W3_d, [HIDDEN, HIDDEN], "w3")
            Wl = cp.tile([HIDDEN, NUM_CLASSES], f32)
            nc.sync.dma_start(Wl[:], Wl_d[:])
            b1 = cp.tile([HIDDEN, 1], f32)
            nc.sync.dma_start(b1[:], b1_d[:])
            b2 = cp.tile([HIDDEN, 1], f32)
            nc.sync.dma_start(b2[:], b2_d[:])
            b3 = cp.tile([HIDDEN, 1], f32)
            nc.sync.dma_start(b3[:], b3_d[:])

            u_of = meta["u_of"]

            # gather chunk in tiles of 128 descriptors; 4 SWDGE queues
            # round-robin, ~2 in flight per queue
            CH = int(os.environ.get("GCN_CH", "8"))
            NSWQ = nswq
            GBUFS = int(os.environ.get("GCN_GBUFS", "8"))
            MLOOK = int(os.environ.get("GCN_MLOOK", "2"))
            MRES = int(os.environ.get("GCN_MRES", "20"))   # M blocks resident
            MBUFS = int(os.environ.get("GCN_MBUFS", "6"))  # streamed-M ring
            NOAG = os.environ.get("GCN_NOAG", "0") == "1"    # ablation
            NOGAT = os.environ.get("GCN_NOGAT", "0") == "1"  # ablation
            qctr = [0]

            # M blocks [0, MRES) stay SBUF-resident for the whole inference
            # (M is layer-invariant): loaded once, reused 3x per rep, and
            # layer starts after an AllGather don't wait on M DMA.
            if MRES > 0:
                mres = cp.tile([128, MRES, ntile, 128], f8)
                nc.sync.dma_start(
                    mres[:].bitcast(u8),
                    mall_d[0:MRES].rearrange("b (p t d) -> p b t d", p=128,
                                             t=ntile))

            # streamed M ring, shared across layers so the next layer's
            # tiles can prefetch during the previous layer's AllGather
            mstream = {}

            def load_mb(lnum, b):
                if b >= NBLK or b < MRES or (lnum, b) in mstream:
                    return
                t = wp.tile([128, ntile, 128], f8, tag="mb",
                            bufs=MBUFS, name=f"mb_{lnum}_{b}")
                nc.sync.dma_start(
                    t[:].bitcast(u8),
                    mall_d[b].rearrange("(p t d) -> p t d", p=128,
                                        t=ntile))
                mstream[(lnum, b)] = t

            def mb_ap(lnum, b):
                if b < MRES:
                    return mres[:, b]
                return mstream[(lnum, b)][:]

            def layer(lnum, tbl, W_sb, b_sb, s_cur, s_nxt, ag_in, ag_out,
                      acc_ps, nxt_lnum=None):
                role = (lnum - 1) % 3 + 1
                nA, nB = NBLK * T_A, NBLK * T_B
                aCH = [(s, min(s + CH, nA)) for s in range(0, nA, CH)]
                bCH = [(s, min(s + CH, nB)) for s in range(0, nB, CH)]
                ga, gb = {}, {}
                ai = bi = 0

                def tail(b, pz):
                    # node-major tail: ACT + PE only (no DVE - it would lock
                    # GpSimd out of the SBUF ports SWDGE needs)
                    dv = dinv_nb[:, b:b + 1]
                    ztN = wp.tile([128, HIDDEN], bf16, tag="ztN", bufs=2,
                                  name=f"ztN_{lnum}_{b}")
                    nc.scalar.activation(ztN[:], pz[:],
                                         mybir.ActivationFunctionType.Copy,
                                         scale=dv)
                    ztT = ps.tile([HIDDEN, 128], bf16, tag="pt16", bufs=2,
                                  name=f"ztT_{lnum}_{b}")
                    nc.tensor.transpose(ztT[:], ztN[:], identm[:])
                    ztF = wp.tile([HIDDEN, 128], bf16, tag="ztF", bufs=2,
                                  name=f"ztF_{lnum}_{b}")
                    nc.scalar.copy(ztF[:], ztT[:])
                    pxn = ps.tile([HIDDEN, 128], f32, tag="pz", bufs=3,
                                  name=f"pxn_{lnum}_{b}")
                    nc.tensor.matmul(pxn[:], lhsT=W_sb[:], rhs=ztF[:],
                                     start=True, stop=True)
                    if role < 3:
                        xh = wp.tile([HIDDEN, 128], bf16, tag="xh", bufs=2,
                                     name=f"xh_{lnum}_{b}")
                        nc.scalar.activation(xh[:], pxn[:],
                                             mybir.ActivationFunctionType.Relu,
                                             bias=b_sb[:])
                        ptr = ps.tile([128, HIDDEN], bf16, tag="pt16", bufs=2,
                                      name=f"ptr_{lnum}_{b}")
                        nc.tensor.transpose(ptr[:], xh[:], identm[:])
                        nc.scalar.activation(s_nxt[:, b, :], ptr[:],
                                             mybir.ActivationFunctionType.Copy,
                                             scale=dv)
                        nc.sync.dma_start(
                            ag_in[b * BLK:(b + 1) * BLK, :],
                            s_nxt[:, b, :].bitcast(f32))
                    else:
                        h3 = wp.tile([HIDDEN, 128], f32, tag="xh", bufs=2,
                                     name=f"h3_{b}")
                        nc.scalar.activation(h3[:], pxn[:],
                                             mybir.ActivationFunctionType.Identity,
                                             bias=b_sb[:])
                        ptr = ps.tile([128, HIDDEN], f32, tag="ptr", bufs=2,
                                      name=f"ptr3_{b}")
                        nc.tensor.transpose(ptr[:], h3[:], ident[:])
                        tr = wp.tile([128, HIDDEN], f32, tag="tr", bufs=2,
                                     name=f"tr3_{b}")
                        nc.scalar.copy(tr[:], ptr[:])
                        bt = wp.tile([128, BLK], f32, tag="bt", bufs=4,
                                     name=f"bt_{b}")
                        nc.sync.dma_start(bt[:],
                                          Bmat_d[:, b * BLK:(b + 1) * BLK])
                        u = int(u_of[b])
                        nc.tensor.matmul(acc_ps[:, u:u + BLK], lhsT=tr[:],
                                         rhs=bt[:], start=False, stop=True)

                LA = int(os.environ.get("GCN_LA", "2"))  # gather issue-ahead
                for b0 in range(MLOOK):
                    load_mb(lnum, MRES + b0)
                pending = None
                for b in range(NBLK):
                    load_mb(lnum, b + MLOOK if b + MLOOK >= MRES else MRES)
                    while ai < len(aCH) and aCH[ai][0] < (b + LA) * T_A:
                        s, e = aCH[ai]
                        gt = wp.tile([128, e - s, TW], f32, tag="gA",
                                     bufs=GBUFS, name=f"gA_{lnum}_{ai}")
                        if not NOGAT:
                            nc.gpsimd.dma_gather(
                                gt[:], tbl[0:32768, :], idx_sb[:, s * 8:e * 8],
                                (e - s) * BLK, (e - s) * BLK, TW,
                                queue_num=qctr[0] % NSWQ)
                        qctr[0] += 1
                        ga[ai] = gt
                        ai += 1
                    while bi < len(bCH) and bCH[bi][0] < (b + LA) * T_B:
                        s, e = bCH[bi]
                        gt = wp.tile([128, e - s, TW], f32, tag="gB",
                                     bufs=GBUFS, name=f"gB_{lnum}_{bi}")
                        if not NOGAT:
                            nc.gpsimd.dma_gather(
                                gt[:], tbl[HI_BASE:NPAD, :],
                                idx_sb[:, nA * 8 + s * 8:nA * 8 + e * 8],
                                (e - s) * BLK, (e - s) * BLK, TW,
                                queue_num=qctr[0] % NSWQ)
                        qctr[0] += 1
                        gb[bi] = gt
                        bi += 1
                    pz = ps.tile([128, HIDDEN], f32, tag="pz", bufs=3,
                                 name=f"pz_{lnum}_{b}")
                    nt = 0
                    for run, gmap, T_r, col0 in (
                        (0, ga, T_A, b * T_A),
                        (1, gb, T_B, b * T_B),
                    ):
                        for t in range(T_r):
                            j = col0 + t                  # stream tile index
                            chunk, sl = j // CH, j % CH
                            nc.tensor.matmul(
                                pz[:], lhsT=mb_ap(lnum, b)[:, nt, :],
                                rhs=gmap[chunk][:].bitcast(bf16)[:, sl, :],
                                start=(nt == 0), stop=False)
                            nt += 1
                    # self-loop: pz[d, f] += self[d, f] via identity lhsT
                    nc.tensor.matmul(pz[:], lhsT=identm[:],
                                     rhs=s_cur[:, b, :],
                                     start=False, stop=True)
                    if pending is not None:
                        tail(*pending)
                    pending = (b, pz)
                if pending is not None:
                    tail(*pending)

                if role < 3:
                    # prefetch the next layer's streamed M into the ring so
                    # those DMAs overlap the AllGather instead of serializing
                    # after it
                    if nxt_lnum is not None:
                        for pb in range(MRES, min(MRES + MBUFS - 1, NBLK)):
                            load_mb(nxt_lnum, pb)
                    if NOAG:
                        nc.sync.dma_start(ag_out[0:SHARD, :], ag_in[:])
                    else:
                        nc.gpsimd.collective_compute(
                            "AllGather", mybir.AluOpType.bypass,
                            replica_groups=[list(range(NCORES))],
                            ins=[ag_in[:]], outs=[ag_out[:]])

            ag_in1 = dp.tile([SHARD, TW], f32)
            ag_in2 = dp.tile([SHARD, TW], f32)
            gidx_sb = cp.tile([128, 4], mybir.dt.int32)
            nc.sync.dma_start(gidx_sb[:], gidx_d[:])
            biasm_sb = cp.tile([NUM_CLASSES, ACC_S], f32)
            nc.sync.dma_start(biasm_sb[:], biasmat_d[:])
            pd_ab = [dp.tile([PD_ROWS, NUM_CLASSES], f32, tag=f"pd{i}",
                             name=f"pd{i}") for i in range(2)]
            zt19 = wp.tile([128, PD_ROWS * NUM_CLASSES // 128], f32)
            nc.vector.memset(zt19[:], 0.0)

            for rep in range(repeat):
              ln1, ln2, ln3 = 3 * rep + 1, 3 * rep + 2, 3 * rep + 3
              ag1_out = dp.tile([NPAD, TW], f32, addr_space="Shared",
                                name=f"ag1_out_{rep}", tag=f"ag1_{rep}")
              ag2_out = dp.tile([NPAD, TW], f32, addr_space="Shared",
                                name=f"ag2_out_{rep}", tag=f"ag2_{rep}")
              # pooling accumulator lives in PSUM; the layer-3 tail matmuls
              # accumulate into it (start=False), so zero it first (the
              # early-epilogue accT copy frees it well before the next rep)
              acc_ps = ps.tile([128, ACC_S], f32, tag="acc", bufs=1,
                               name=f"accps_{rep}")
              nc.vector.memset(acc_ps[:], 0.0)
              # alternate pooling scatter buffers so rep r+1's zeroing does
              # not wait on rep r's AllReduce read (WAR)
              pd = pd_ab[rep % 2]
              layer(ln1, table1, W1p, b1, selfA, selfB, ag_in1, ag1_out, None,
                    nxt_lnum=ln2)
              layer(ln2, ag1_out, W2, b2, selfB, selfA, ag_in2, ag2_out, None,
                    nxt_lnum=ln3)
              layer(ln3, ag2_out, W3, b3, selfA, selfB, None, None, acc_ps)

              # ---- pooling epilogue (gathers are done; DVE is safe) ----
              nc.sync.dma_start(
                pd[:].rearrange("a b -> (a b)").rearrange("(p f) -> p f", p=128),
                zt19[:])
              accT = wp.tile([128, ACC_S], f32, tag="accsb", bufs=2,
                             name=f"accsb_{rep}")
              nc.scalar.copy(accT[:], acc_ps[:])

              for k in range(4):
                py = ps.tile([NUM_CLASSES, 128], f32, tag="ptr", bufs=2,
                             name=f"py_{rep}_{k}")
                nc.tensor.matmul(py[:], lhsT=Wl[:],
                                 rhs=accT[:, k * 128:(k + 1) * 128],
                                 start=True, stop=True)
                y = wp.tile([NUM_CLASSES, 128], f32, tag="ye", bufs=2,
                            name=f"y_{rep}_{k}")
                nc.vector.tensor_tensor(
                    out=y[:], in0=py[:],
                    in1=biasm_sb[:, k * 128:(k + 1) * 128],
                    op=mybir.AluOpType.add)
                pyt = ps.tile([128, NUM_CLASSES], f32, tag="ptr", bufs=2,
                              name=f"pyt_{rep}_{k}")
                nc.tensor.transpose(pyt[:], y[:],
                                    ident[:NUM_CLASSES, :NUM_CLASSES])
                yT = wp.tile([128, NUM_CLASSES], f32, tag="yt2", bufs=2,
                             name=f"yT_{rep}_{k}")
                nc.scalar.copy(yT[:], pyt[:])
                nc.gpsimd.indirect_dma_start(
                    out=pd[:],
                    out_offset=bass.IndirectOffsetOnAxis(ap=gidx_sb[:, k:k + 1],
                                                         axis=0),
                    in_=yT[:], in_offset=None)

              pd_red = dp.tile([PD_ROWS, NUM_CLASSES], f32,
                               addr_space="Shared", name=f"pd_red_{rep}",
                               tag=f"pdr_{rep}")
              nc.gpsimd.collective_compute(
                "AllReduce", mybir.AluOpType.add,
                replica_groups=[list(range(NCORES))],
                ins=[pd[:]], outs=[pd_red[:]])
              nc.sync.dma_start(out_d[:], pd_red[0:NUM_GRAPHS, :])

    nc.compile()
    return nc


# --------------------------------------------------------------------------
def kernel(**inputs):
    from concourse import bass_utils

    meta, per_core = _preprocess(**inputs)
    key = (meta["T_A"], meta["T_B"])
    if key not in _cache:
        _cache[key] = _build(meta)
    nc = _cache[key]
    res = bass_utils.run_bass_kernel_spmd(nc, per_core,
                                          core_ids=list(range(NCORES)))
    return np.asarray(res.results[0]["out"], np.float32)



# revision 25
# speedup vs baseline: 1.1778x; 1.0741x over previous
"""GCN (3x GCNConv + global_mean_pool + linear) on 8 Trainium2 NeuronCores.

Self-contained: hardcoded problem shapes (N=50000, E=800000, H=128, F_IN=11,
G=2048).

Math (per conv layer, PyG GCNConv):
    z[d] = dinv[d] * ( sum_{e:dst=d} dinv[src_e]*x[src_e]  +  dinv[d]*x[d] )
    x' = relu(z @ W + b)          (no relu on layer 3)
with dinv = 1/sqrt(1+indeg). The feature table is pre-scaled by dinv (x~ =
dinv*x) so edge contributions need only the dst-side dinv, applied per
128-node block after PSUM accumulation.

Distribution: nodes (padded to 50176 = 8*49*128) sharded contiguously across
8 cores; each core aggregates its own dst blocks, gathering source rows from
a replicated feature table (AllGather per layer). Pooling partials are
scatter-written to graph rows and AllReduced.

Device pipeline per layer/core (per dst block b, tails pipelined one block
behind the aggregation matmuls):
  dma_gather src rows (1024-desc chunks = GCN_CH=8 tiles, f32-typed 256B
  rows carrying bf16 payload - "V16") -> PE: pzN[dst,feat] += M_tile^T @
  G_tile (lhsT = host-precomputed one-hot fp8 M, SBUF-resident for blocks
  < GCN_MRES and DRAM-streamed/prefetched otherwise, rhs = gathered tile
  bitcast bf16) + an identity matmul adds the node-major self-loop tile ->
  ACT copy pzN*dinv -> PE transpose -> ACT copy -> PE @W -> ACT relu+bias ->
  PE transpose -> ACT copy*dinv into the next layer's self tiles -> DMA to
  shard -> AllGather.

Critical performance facts (HW-measured):
- SWDGE descriptor generation bounds the kernel. 4 SWDGE queues
  (num_swdge_queues=4 = ucode MAX, queue_num round-robin). Per-call fixed
  overhead ~1us makes FEWER, BIGGER dma_gather calls win: descriptor ring
  capacity scales with dynamic_dma_scratch_size (ring descs/queue ~=
  scratch/16). scratch=32768 supports 1024-desc calls x2 in flight
  (GCN_CH=8); scratch=49152 + 1536-desc calls HANGS the ucode
  (await_space never satisfied - ring scaling is NOT linear past 32768),
  as does 2048-desc at scratch=16384. A wedged run needs a fresh process
  (+NEURON_RT_RESET_CORES=1).
- bf16-typed gathers are SLOWER per descriptor than f32, hence the "bf16
  payload viewed as f32 half-width" trick with bitcast at the PE.
- One-hot M matrices ship as fp8 e4m3 raw bytes (0x38 == 1.0): exact
  numerics, half the HBM traffic; PE supports mixed fp8 lhsT x bf16 rhs.
  M is layer-invariant: blocks [0, GCN_MRES) stay SBUF-resident all rep;
  the streamed remainder prefetches into the AllGather windows.
- ANY 2-input DVE op locks the SBUF port pair GpSimd needs to write SWDGE
  descriptors (exclusive lock, full block) - overlapped DVE work stretches
  the gathers ~3x. The hot path must use only PE/ACT/DMA: all per-block
  scaling uses ACT `activation(scale=per-partition dinv)` in node-major
  layout; pooling accumulates in PSUM. DVE appears only in startup consts
  and the tiny per-inference epilogue.
- The 8-core AllGather costs only tens of us; don't bother splitting it.
- Same-tag const-pool tiles share ONE slot ring: distinct tags per weight
  are REQUIRED or repeat>=2 timing builds deadlock the tile scheduler.
"""
import sys

sys.path.insert(0, "/opt/trn_rl_repo")

import numpy as np

N_NODES = 50000
N_EDGES = 800000
HIDDEN = 128
F_IN = 11
F1 = 128                   # layer-1 feature padding (bf16 -> 256B rows)
NUM_CLASSES = 19
NUM_GRAPHS = 2048
NCORES = 8
BLK = 128
NBLK = 49                  # blocks per core
SHARD = NBLK * BLK         # 6272 nodes per core
NPAD = NCORES * SHARD      # 50176
TW = F1 // 2               # table row width in f32 elements (bf16 payload)
LO_END = 17408             # A-window: table[0:32768), idx=src
HI_BASE = NPAD - 32768     # 17408; B-window: table[17408:50176), idx=src-HI_BASE
ACC_S = 512                # core-relative pooling slots (4 tiles of 128)
PD_ROWS = 2176             # padded graph rows for scatter (>=2048, *19 %128==0)

_cache = {}


# --------------------------------------------------------------------------
# host preprocessing
# --------------------------------------------------------------------------
def _preprocess(x, edge_index, batch, W1, b1, W2, b2, W3, b3, Wl, bl):
    import ml_dtypes
    bfnp = ml_dtypes.bfloat16

    src = np.asarray(edge_index[0], dtype=np.int64)
    dst = np.asarray(edge_index[1], dtype=np.int64)
    batch = np.asarray(batch, dtype=np.int64)
    x = np.asarray(x, np.float32)

    x_pad = np.zeros((NPAD, F_IN), np.float32)
    x_pad[:N_NODES] = x
    batch_pad = np.full(NPAD, -1, np.int64)
    batch_pad[:N_NODES] = batch

    # --- in-degree-balanced node permutation within 12-block windows -------
    # (keeps pooling graph-windows narrow while equalizing per-block edge
    #  counts so the uniform SPMD tile budgets waste fewer gather slots)
    indeg = np.bincount(dst, minlength=NPAD).astype(np.int64)
    indeg_lo = np.bincount(dst[src < LO_END], minlength=NPAD).astype(np.int64)
    indeg_hi = np.bincount(dst[src >= 32768], minlength=NPAD).astype(np.int64)
    perm = np.arange(NPAD)
    import os
    W = int(os.environ.get("GCN_BALW", "12"))
    for c in range(NCORES if os.environ.get("GCN_BAL", "1") == "1" else 0):
        for w0 in range(0, NBLK, W):
            nb = min(W, NBLK - w0)
            p0 = c * SHARD + w0 * BLK
            ids = perm[p0:p0 + nb * BLK].copy()
            tot, lo, hi = indeg[ids], indeg_lo[ids], indeg_hi[ids]
            at = max(tot.sum() / nb, 1.0)
            al = max(lo.sum() / nb, 1.0)
            ah = max(hi.sum() / nb, 1.0)
            order = np.argsort(-tot, kind="stable")
            bt = np.zeros(nb)
            blo = np.zeros(nb)
            bhi = np.zeros(nb)
            bcnt = np.zeros(nb, np.int64)
            assign = np.empty(nb * BLK, np.int64)
            for i in order:
                scr = np.maximum(
                    np.maximum((bt + tot[i]) / at, (blo + lo[i]) / al),
                    (bhi + hi[i]) / ah)
                scr[bcnt >= BLK] = np.inf
                b = int(np.argmin(scr))
                assign[i] = b
                bt[b] += tot[i]
                blo[b] += lo[i]
                bhi[b] += hi[i]
                bcnt[b] += 1
            perm[p0:p0 + nb * BLK] = np.concatenate(
                [ids[assign == b] for b in range(nb)])
    inv = np.empty(NPAD, np.int64)
    inv[perm] = np.arange(NPAD)
    src = inv[src]
    dst = inv[dst]
    x_pad = x_pad[perm]
    batch_pad = batch_pad[perm]

    deg = 1.0 + np.bincount(dst, minlength=NPAD).astype(np.float32)
    dinv_pad = (1.0 / np.sqrt(deg)).astype(np.float32)
    # per-node dinv laid out [node-in-block, NBLK] per core
    dinv_nb = np.ascontiguousarray(
        dinv_pad.reshape(NCORES, NBLK, BLK).transpose(0, 2, 1))

    xs = (x_pad * dinv_pad[:, None]).astype(np.float32)   # x~ = dinv * x
    # bf16 payload disguised as f32 rows of width F1//2: gather moves 256B
    # rows at the (faster) f32 descriptor rate; PE reads via bitcast(bf16)
    t1b = np.zeros((NPAD, F1), bfnp)
    t1b[:, :F_IN] = xs.astype(bfnp)
    table1 = t1b.view(np.float32)                         # [NPAD, TW]
    # layer-1 self-loop tiles, node-major per shard (same bf16-as-f32 view)
    self1 = np.ascontiguousarray(
        t1b.view(np.float32).reshape(NCORES, SHARD, TW))  # [C, SHARD, TW]

    # --- edge grouping -----------------------------------------------------
    core_of = dst // SHARD
    blk_of = (dst % SHARD) // BLK
    rel_of = (dst % BLK).astype(np.int64)
    gblk = core_of * NBLK + blk_of
    cls = np.where(src < LO_END, 0, np.where(src >= 32768, 2, 1)).astype(np.int8)

    nblk_g = NCORES * NBLK
    n_lo = np.bincount(gblk[cls == 0], minlength=nblk_g)
    n_mid = np.bincount(gblk[cls == 1], minlength=nblk_g)
    n_hi = np.bincount(gblk[cls == 2], minlength=nblk_g)

    T_A = max(1, int(np.max(-(-n_lo // BLK))))
    a_fill = np.minimum(n_mid, T_A * BLK - n_lo)
    T_B = max(1, int(np.max(-(-(n_hi + n_mid - a_fill) // BLK))))
    ntile = T_A + T_B
    slots_core = NBLK * ntile * BLK

    order = np.lexsort((cls, gblk))
    src_o, rel_o, cls_o = src[order], rel_of[order], cls[order]
    blk_starts = np.searchsorted(gblk[order], np.arange(nblk_g + 1))

    idx_all = np.zeros((NCORES, slots_core), np.int16)
    dstrel_all = np.full((NCORES, slots_core), 255, np.int64)
    for c in range(NCORES):
        for run in (0, 1):
            T_r = T_A if run == 0 else T_B
            base0 = 0 if run == 0 else NBLK * T_A * BLK
            for b in range(NBLK):
                g = c * NBLK + b
                s, e = blk_starts[g], blk_starts[g + 1]
                bsrc, brel, bcls = src_o[s:e], rel_o[s:e], cls_o[s:e]
                a = int(a_fill[g])
                mid_idx = np.nonzero(bcls == 1)[0]
                if run == 0:
                    sel = np.concatenate([np.nonzero(bcls == 0)[0], mid_idx[:a]])
                    iv = bsrc[sel]
                else:
                    sel = np.concatenate([mid_idx[a:], np.nonzero(bcls == 2)[0]])
                    iv = bsrc[sel] - HI_BASE
                k = len(sel)
                assert k <= T_r * BLK
                pos = base0 + b * T_r * BLK
                idx_all[c, pos:pos + k] = iv.astype(np.int16)
                dstrel_all[c, pos:pos + k] = brel[sel]

    idx16 = np.zeros((NCORES, 128, slots_core // 16), np.int16)
    for c in range(NCORES):
        idx16[c] = np.tile(idx_all[c].reshape(-1, 16).T, (8, 1))

    # host-precomputed one-hot M, per block: [NBLK, 128 slot, ntile, 128 dst]
    # (fp8 e4m3 raw bytes, 0x38 == 1.0; loaded per block as the aggregation
    # matmuls' lhsT — mixed fp8 lhsT x bf16 rhs is a supported PE mode and
    # one-hot values are exact in fp8, at half the HBM traffic of bf16)
    mall = np.zeros((NCORES, NBLK, 128, ntile, 128), np.uint8)
    for c in range(NCORES):
        arrA = dstrel_all[c][:NBLK * T_A * BLK].reshape(NBLK, T_A, BLK)
        arrB = dstrel_all[c][NBLK * T_A * BLK:].reshape(NBLK, T_B, BLK)
        for arr, t0 in ((arrA, 0), (arrB, T_A)):
            bi, ti, pi = np.nonzero(arr <= 127)
            mall[c, bi, pi, t0 + ti, arr[bi, ti, pi]] = 0x38

    # --- pooling -----------------------------------------------------------
    cnt = np.bincount(batch, minlength=NUM_GRAPHS).astype(np.float32)
    inv_cnt = (1.0 / np.maximum(cnt, 1.0)).astype(np.float32)
    bp = batch_pad.reshape(NCORES, SHARD)
    gc_lo = np.array([int(bp[c][bp[c] >= 0].min()) for c in range(NCORES)])

    # uniform (SPMD) core-relative window base per block: cover all cores
    lo_need = np.full(NBLK, 10 ** 9, np.int64)
    hi_need = np.full(NBLK, 0, np.int64)
    for c in range(NCORES):
        for b in range(NBLK):
            nodes = bp[c, b * BLK:(b + 1) * BLK]
            real = nodes[nodes >= 0]
            if len(real):
                lo_need[b] = min(lo_need[b], real.min() - gc_lo[c])
                hi_need[b] = max(hi_need[b], real.max() - gc_lo[c])
    u_of = np.clip(lo_need, 0, ACC_S - BLK)
    assert (hi_need - u_of).max() < BLK and hi_need.max() < ACC_S

    Bmat = np.zeros((NCORES, 128, NBLK * BLK), np.float32)
    for c in range(NCORES):
        for b in range(NBLK):
            nodes = bp[c, b * BLK:(b + 1) * BLK]
            p = np.nonzero(nodes >= 0)[0]
            if len(p) == 0:
                continue
            s = nodes[p] - gc_lo[c] - u_of[b]
            assert (s >= 0).all() and (s < BLK).all(), (c, b, s.min(), s.max())
            Bmat[c, p, b * BLK + s] = inv_cnt[nodes[p]]

    # absolute graph row per core-relative slot; dummies -> pad rows
    gidx = np.zeros((NCORES, 128, 4), np.int32)
    covered = np.zeros((NCORES, ACC_S), bool)
    for c in range(NCORES):
        for k in range(4):
            g_abs = gc_lo[c] + k * 128 + np.arange(128)
            ok = g_abs < NUM_GRAPHS
            gidx[c, :, k] = np.where(ok, g_abs, 2100)
            covered[c, k * 128:(k + 1) * 128] = ok

    # bias: designate exactly one (core, slot) per graph
    biasmat = np.zeros((NCORES, NUM_CLASSES, ACC_S), np.float32)
    bl32 = np.asarray(bl, np.float32)
    done = np.zeros(NUM_GRAPHS, bool)
    for c in range(NCORES):
        for sl in range(ACC_S):
            if covered[c, sl]:
                g = gc_lo[c] + sl
                if not done[g]:
                    done[g] = True
                    biasmat[c, :, sl] = bl32
    assert done.all()

    W1pf = np.zeros((F1, HIDDEN), np.float32)  # rows >= F_IN stay zero
    W1pf[:F_IN] = np.asarray(W1, np.float32)
    wts = dict(
        W1p=W1pf.astype(bfnp).view(np.float32),
        W2=np.asarray(W2, np.float32).astype(bfnp).view(np.float32),
        W3=np.asarray(W3, np.float32).astype(bfnp).view(np.float32),
        Wl=np.asarray(Wl, np.float32),
        identm=np.eye(128, dtype=bfnp).view(np.float32),
        b1=np.asarray(b1, np.float32).reshape(HIDDEN, 1),
        b2=np.asarray(b2, np.float32).reshape(HIDDEN, 1),
        b3=np.asarray(b3, np.float32).reshape(HIDDEN, 1),
    )

    meta = dict(T_A=T_A, T_B=T_B, slots_core=slots_core, u_of=u_of)
    per_core = [dict(idx16=idx16[c], mall=mall[c].reshape(NBLK, -1),
                     self1=self1[c], dinv_nb=dinv_nb[c], Bmat=Bmat[c],
                     gidx=gidx[c], biasmat=biasmat[c], table1=table1, **wts)
                for c in range(NCORES)]
    return meta, per_core


# --------------------------------------------------------------------------
# device program
# --------------------------------------------------------------------------
def _build(meta, repeat=1):
    import concourse.bacc as bacc
    import concourse.bass as bass
    import concourse.tile as tile
    from concourse import mybir
    from concourse.masks import make_identity

    T_A, T_B = meta["T_A"], meta["T_B"]
    slots = meta["slots_core"]
    ntile = T_A + T_B
    f32 = mybir.dt.float32
    bf16 = mybir.dt.bfloat16
    f8 = mybir.dt.float8e4
    u8 = mybir.dt.uint8

    import os
    scr = int(os.environ.get("GCN_SCR", "32768"))
    nswq = int(os.environ.get("GCN_NSWQ", "4"))
    nc = bacc.Bacc("TRN2", target_bir_lowering=False, debug=False,
                   num_devices=NCORES, dynamic_dma_scratch_size=scr,
                   num_swdge_queues=nswq)
    ti = lambda n, s, d=f32: nc.dram_tensor(n, s, d, kind="ExternalInput")
    table1 = ti("table1", [NPAD, TW])
    idx16 = ti("idx16", [128, slots // 16], mybir.dt.int16)
    mall_d = ti("mall", [NBLK, 128 * ntile * 128], u8)
    self1_d = ti("self1", [SHARD, TW])
    dinv_d = ti("dinv_nb", [128, NBLK])
    Bmat_d = ti("Bmat", [128, NBLK * BLK])
    gidx_d = ti("gidx", [128, 4], mybir.dt.int32)
    biasmat_d = ti("biasmat", [NUM_CLASSES, ACC_S])
    W1p_d = ti("W1p", [F1, HIDDEN // 2])
    W2_d = ti("W2", [HIDDEN, HIDDEN // 2])
    W3_d = ti("W3", [HIDDEN, HIDDEN // 2])
    identm_d = ti("identm", [128, 64])
    Wl_d = ti("Wl", [HIDDEN, NUM_CLASSES])
    b1_d, b2_d, b3_d = ti("b1", [HIDDEN, 1]), ti("b2", [HIDDEN, 1]), ti("b3", [HIDDEN, 1])
    out_d = nc.dram_tensor("out", [NUM_GRAPHS, NUM_CLASSES], f32,
                           kind="ExternalOutput")

    with tile.TileContext(nc) as tc:
        with (
            tc.tile_pool(name="const", bufs=1) as cp,
            tc.tile_pool(name="work", bufs=1) as wp,
            tc.tile_pool(name="ps", bufs=2, space="PSUM") as ps,
            tc.tile_pool(name="dram", bufs=1, space="DRAM") as dp,
        ):
            # ---- constants / persistent state ----
            idx_sb = cp.tile([128, slots // 16], mybir.dt.int16)
            nc.sync.dma_start(idx_sb[:], idx16[:])
            dinv_nb = cp.tile([128, NBLK], f32)
            nc.sync.dma_start(dinv_nb[:], dinv_d[:])
            # self-loop tiles, node-major bf16: ping-pong across layers
            selfA = cp.tile([128, NBLK, 2 * TW], bf16)
            nc.sync.dma_start(
                selfA[:].bitcast(f32),
                self1_d[:].rearrange("(b p) f -> p b f", p=128))
            selfB = cp.tile([128, NBLK, 2 * TW], bf16)
            ident = cp.tile([128, 128], f32)
            make_identity(nc, ident[:])
            identm = cp.tile([128, 128], bf16)
            nc.sync.dma_start(identm[:].bitcast(f32), identm_d[:])

            def load_w16(dram, shape, tg):
                # distinct tag per weight: same-tag cp tiles share one slot
                # ring (bufs=1), which deadlocks the scheduler when reps>1
                # re-read an early weight after its slot was recycled
                wb = cp.tile(shape, bf16, tag=tg, name=tg)
                nc.sync.dma_start(wb[:].bitcast(f32), dram[:])
                return wb

            W1p = load_w16(W1p_d, [F1, HIDDEN], "w1p")
            W2 = load_w16(W2_d, [HIDDEN, HIDDEN], "w2")
            W3 = load_w16(W3_d, [HIDDEN, HIDDEN], "w3")
            Wl = cp.tile([HIDDEN, NUM_CLASSES], f32)
            nc.sync.dma_start(Wl[:], Wl_d[:])
            b1 = cp.tile([HIDDEN, 1], f32)
            nc.sync.dma_start(b1[:], b1_d[:])
            b2 = cp.tile([HIDDEN, 1], f32)
            nc.sync.dma_start(b2[:], b2_d[:])
            b3 = cp.tile([HIDDEN, 1], f32)
            nc.sync.dma_start(b3[:], b3_d[:])

            u_of = meta["u_of"]

            # gather chunk in tiles of 128 descriptors; 4 SWDGE queues
            # round-robin, ~2 in flight per queue
            CH = int(os.environ.get("GCN_CH", "8"))
            NSWQ = nswq
            GBUFS = int(os.environ.get("GCN_GBUFS", "8"))
            MLOOK = int(os.environ.get("GCN_MLOOK", "2"))
            MRES = int(os.environ.get("GCN_MRES", "24"))   # M blocks resident
            MBUFS = int(os.environ.get("GCN_MBUFS", "6"))  # streamed-M ring
            NOAG = os.environ.get("GCN_NOAG", "0") == "1"    # ablation
            NOGAT = os.environ.get("GCN_NOGAT", "0") == "1"  # ablation
            qctr = [0]

            # M blocks [0, MRES) stay SBUF-resident for the whole inference
            # (M is layer-invariant): loaded once, reused 3x per rep, and
            # layer starts after an AllGather don't wait on M DMA.
            if MRES > 0:
                mres = cp.tile([128, MRES, ntile, 128], f8)
                nc.sync.dma_start(
                    mres[:].bitcast(u8),
                    mall_d[0:MRES].rearrange("b (p t d) -> p b t d", p=128,
                                             t=ntile))

            # streamed M ring, shared across layers so the next layer's
            # tiles can prefetch during the previous layer's AllGather
            mstream = {}

            def load_mb(lnum, b):
                if b >= NBLK or b < MRES or (lnum, b) in mstream:
                    return
                t = wp.tile([128, ntile, 128], f8, tag="mb",
                            bufs=MBUFS, name=f"mb_{lnum}_{b}")
                nc.sync.dma_start(
                    t[:].bitcast(u8),
                    mall_d[b].rearrange("(p t d) -> p t d", p=128,
                                        t=ntile))
                mstream[(lnum, b)] = t

            def mb_ap(lnum, b):
                if b < MRES:
                    return mres[:, b]
                return mstream[(lnum, b)][:]

            def layer(lnum, tbl, W_sb, b_sb, s_cur, s_nxt, ag_in, ag_out,
                      acc_ps, nxt_lnum=None):
                role = (lnum - 1) % 3 + 1
                nA, nB = NBLK * T_A, NBLK * T_B
                aCH = [(s, min(s + CH, nA)) for s in range(0, nA, CH)]
                bCH = [(s, min(s + CH, nB)) for s in range(0, nB, CH)]
                ga, gb = {}, {}
                ai = bi = 0

                def tail(b, pz):
                    # node-major tail: ACT + PE only (no DVE - it would lock
                    # GpSimd out of the SBUF ports SWDGE needs)
                    dv = dinv_nb[:, b:b + 1]
                    ztN = wp.tile([128, HIDDEN], bf16, tag="ztN", bufs=2,
                                  name=f"ztN_{lnum}_{b}")
                    nc.scalar.activation(ztN[:], pz[:],
                                         mybir.ActivationFunctionType.Copy,
                                         scale=dv)
                    ztT = ps.tile([HIDDEN, 128], bf16, tag="pt16", bufs=2,
                                  name=f"ztT_{lnum}_{b}")
                    nc.tensor.transpose(ztT[:], ztN[:], identm[:])
                    ztF = wp.tile([HIDDEN, 128], bf16, tag="ztF", bufs=2,
                                  name=f"ztF_{lnum}_{b}")
                    nc.scalar.copy(ztF[:], ztT[:])
                    pxn = ps.tile([HIDDEN, 128], f32, tag="pz", bufs=3,
                                  name=f"pxn_{lnum}_{b}")
                    nc.tensor.matmul(pxn[:], lhsT=W_sb[:], rhs=ztF[:],
                                     start=True, stop=True)
                    if role < 3:
                        xh = wp.tile([HIDDEN, 128], bf16, tag="xh", bufs=2,
                                     name=f"xh_{lnum}_{b}")
                        nc.scalar.activation(xh[:], pxn[:],
                                             mybir.ActivationFunctionType.Relu,
                                             bias=b_sb[:])
                        ptr = ps.tile([128, HIDDEN], bf16, tag="pt16", bufs=2,
                                      name=f"ptr_{lnum}_{b}")
                        nc.tensor.transpose(ptr[:], xh[:], identm[:])
                        nc.scalar.activation(s_nxt[:, b, :], ptr[:],
                                             mybir.ActivationFunctionType.Copy,
                                             scale=dv)
                        nc.sync.dma_start(
                            ag_in[b * BLK:(b + 1) * BLK, :],
                            s_nxt[:, b, :].bitcast(f32))
                    else:
                        h3 = wp.tile([HIDDEN, 128], f32, tag="xh", bufs=2,
                                     name=f"h3_{b}")
                        nc.scalar.activation(h3[:], pxn[:],
                                             mybir.ActivationFunctionType.Identity,
                                             bias=b_sb[:])
                        ptr = ps.tile([128, HIDDEN], f32, tag="ptr", bufs=2,
                                      name=f"ptr3_{b}")
                        nc.tensor.transpose(ptr[:], h3[:], ident[:])
                        tr = wp.tile([128, HIDDEN], f32, tag="tr", bufs=2,
                                     name=f"tr3_{b}")
                        nc.scalar.copy(tr[:], ptr[:])
                        bt = wp.tile([128, BLK], f32, tag="bt", bufs=4,
                                     name=f"bt_{b}")
                        nc.sync.dma_start(bt[:],
                                          Bmat_d[:, b * BLK:(b + 1) * BLK])
                        u = int(u_of[b])
                        nc.tensor.matmul(acc_ps[:, u:u + BLK], lhsT=tr[:],
                                         rhs=bt[:], start=False, stop=True)

                LA = int(os.environ.get("GCN_LA", "3"))  # gather issue-ahead
                for b0 in range(MLOOK):
                    load_mb(lnum, MRES + b0)
                pending = None
                for b in range(NBLK):
                    load_mb(lnum, b + MLOOK if b + MLOOK >= MRES else MRES)
                    while ai < len(aCH) and aCH[ai][0] < (b + LA) * T_A:
                        s, e = aCH[ai]
                        gt = wp.tile([128, e - s, TW], f32, tag="gA",
                                     bufs=GBUFS, name=f"gA_{lnum}_{ai}")
                        if not NOGAT:
                            nc.gpsimd.dma_gather(
                                gt[:], tbl[0:32768, :], idx_sb[:, s * 8:e * 8],
                                (e - s) * BLK, (e - s) * BLK, TW,
                                queue_num=qctr[0] % NSWQ)
                        qctr[0] += 1
                        ga[ai] = gt
                        ai += 1
                    while bi < len(bCH) and bCH[bi][0] < (b + LA) * T_B:
                        s, e = bCH[bi]
                        gt = wp.tile([128, e - s, TW], f32, tag="gB",
                                     bufs=GBUFS, name=f"gB_{lnum}_{bi}")
                        if not NOGAT:
                            nc.gpsimd.dma_gather(
                                gt[:], tbl[HI_BASE:NPAD, :],
                                idx_sb[:, nA * 8 + s * 8:nA * 8 + e * 8],
                                (e - s) * BLK, (e - s) * BLK, TW,
                                queue_num=qctr[0] % NSWQ)
                        qctr[0] += 1
                        gb[bi] = gt
                        bi += 1
                    pz = ps.tile([128, HIDDEN], f32, tag="pz", bufs=3,
                                 name=f"pz_{lnum}_{b}")
                    nt = 0
                    for run, gmap, T_r, col0 in (
                        (0, ga, T_A, b * T_A),
                        (1, gb, T_B, b * T_B),
                    ):
                        for t in range(T_r):
                            j = col0 + t                  # stream tile index
                            chunk, sl = j // CH, j % CH
                            nc.tensor.matmul(
                                pz[:], lhsT=mb_ap(lnum, b)[:, nt, :],
                                rhs=gmap[chunk][:].bitcast(bf16)[:, sl, :],
                                start=(nt == 0), stop=False)
                            nt += 1
                    # self-loop: pz[d, f] += self[d, f] via identity lhsT
                    nc.tensor.matmul(pz[:], lhsT=identm[:],
                                     rhs=s_cur[:, b, :],
                                     start=False, stop=True)
                    if pending is not None:
                        tail(*pending)
                    pending = (b, pz)
                if pending is not None:
                    tail(*pending)

                if role < 3:
                    # prefetch the next layer's streamed M into the ring so
                    # those DMAs overlap the AllGather instead of serializing
                    # after it
                    if nxt_lnum is not None:
                        for pb in range(MRES, min(MRES + MBUFS - 1, NBLK)):
                            load_mb(nxt_lnum, pb)
                    if NOAG:
                        nc.sync.dma_start(ag_out[0:SHARD, :], ag_in[:])
                    else:
                        nc.gpsimd.collective_compute(
                            "AllGather", mybir.AluOpType.bypass,
                            replica_groups=[list(range(NCORES))],
                            ins=[ag_in[:]], outs=[ag_out[:]])

            ag_in1 = dp.tile([SHARD, TW], f32)
            ag_in2 = dp.tile([SHARD, TW], f32)
            gidx_sb = cp.tile([128, 4], mybir.dt.int32)
            nc.sync.dma_start(gidx_sb[:], gidx_d[:])
            biasm_sb = cp.tile([NUM_CLASSES, ACC_S], f32)
            nc.sync.dma_start(biasm_sb[:], biasmat_d[:])
            pd_ab = [dp.tile([PD_ROWS, NUM_CLASSES], f32, tag=f"pd{i}",
                             name=f"pd{i}") for i in range(2)]
            zt19 = wp.tile([128, PD_ROWS * NUM_CLASSES // 128], f32)
            nc.vector.memset(zt19[:], 0.0)

            for rep in range(repeat):
              ln1, ln2, ln3 = 3 * rep + 1, 3 * rep + 2, 3 * rep + 3
              ag1_out = dp.tile([NPAD, TW], f32, addr_space="Shared",
                                name=f"ag1_out_{rep}", tag=f"ag1_{rep}")
              ag2_out = dp.tile([NPAD, TW], f32, addr_space="Shared",
                                name=f"ag2_out_{rep}", tag=f"ag2_{rep}")
              # pooling accumulator lives in PSUM; the layer-3 tail matmuls
              # accumulate into it (start=False), so zero it first (the
              # early-epilogue accT copy frees it well before the next rep)
              acc_ps = ps.tile([128, ACC_S], f32, tag="acc", bufs=1,
                               name=f"accps_{rep}")
              nc.vector.memset(acc_ps[:], 0.0)
              # alternate pooling scatter buffers so rep r+1's zeroing does
              # not wait on rep r's AllReduce read (WAR)
              pd = pd_ab[rep % 2]
              layer(ln1, table1, W1p, b1, selfA, selfB, ag_in1, ag1_out, None,
                    nxt_lnum=ln2)
              layer(ln2, ag1_out, W2, b2, selfB, selfA, ag_in2, ag2_out, None,
                    nxt_lnum=ln3)
              layer(ln3, ag2_out, W3, b3, selfA, selfB, None, None, acc_ps)

              # ---- pooling epilogue (gathers are done; DVE is safe) ----
              nc.sync.dma_start(
                pd[:].rearrange("a b -> (a b)").rearrange("(p f) -> p f", p=128),
                zt19[:])
              accT = wp.tile([128, ACC_S], f32, tag="accsb", bufs=2,
                             name=f"accsb_{rep}")
              nc.scalar.copy(accT[:], acc_ps[:])

              for k in range(4):
                py = ps.tile([NUM_CLASSES, 128], f32, tag="ptr", bufs=2,
                             name=f"py_{rep}_{k}")
                nc.tensor.matmul(py[:], lhsT=Wl[:],
                                 rhs=accT[:, k * 128:(k + 1) * 128],
                                 start=True, stop=True)
                y = wp.tile([NUM_CLASSES, 128], f32, tag="ye", bufs=2,
                            name=f"y_{rep}_{k}")
                nc.vector.tensor_tensor(
                    out=y[:], in0=py[:],
                    in1=biasm_sb[:, k * 128:(k + 1) * 128],
                    op=mybir.AluOpType.add)
                pyt = ps.tile([128, NUM_CLASSES], f32, tag="ptr", bufs=2,
                              name=f"pyt_{rep}_{k}")
                nc.tensor.transpose(pyt[:], y[:],
                                    ident[:NUM_CLASSES, :NUM_CLASSES])
                yT = wp.tile([128, NUM_CLASSES], f32, tag="yt2", bufs=2,
                             name=f"yT_{rep}_{k}")
                nc.scalar.copy(yT[:], pyt[:])
                nc.gpsimd.indirect_dma_start(
                    out=pd[:],
                    out_offset=bass.IndirectOffsetOnAxis(ap=gidx_sb[:, k:k + 1],
                                                         axis=0),
                    in_=yT[:], in_offset=None)

              pd_red = dp.tile([PD_ROWS, NUM_CLASSES], f32,
                               addr_space="Shared", name=f"pd_red_{rep}",
                               tag=f"pdr_{rep}")
              nc.gpsimd.collective_compute(
                "AllReduce", mybir.AluOpType.add,
                replica_groups=[list(range(NCORES))],
                ins=[pd[:]], outs=[pd_red[:]])
              nc.sync.dma_start(out_d[:], pd_red[0:NUM_GRAPHS, :])

    nc.compile()
    return nc


# --------------------------------------------------------------------------
def kernel(**inputs):
    from concourse import bass_utils

    meta, per_core = _preprocess(**inputs)
    key = (meta["T_A"], meta["T_B"])
    if key not in _cache:
        _cache[key] = _build(meta)
    nc = _cache[key]
    res = bass_utils.run_bass_kernel_spmd(nc, per_core,
                                          core_ids=list(range(NCORES)))
    return np.asarray(res.results[0]["out"], np.float32)

